# revision 21
# baseline (speedup 1.0000x reference)
"""Trainium2 Bass kernel for the ChebConv GNN problem
(nn_ChebConvConvolutional): 2x GCNConv + 1x ChebConv(K=3), N=10000 nodes,
E=160000 edges, F=512, celu activations.

Strategy (8 NeuronCores, SPMD):
  * Nodes are sharded 1250/core (padded to 1280). Edges are sharded by
    destination core and grouped into 128-dest tiles; per dest-tile the
    source nodes are deduplicated and the edge weights are baked into dense
    [128 src x 128 dst] one-hot "S" matrices (GCN self-loops folded in as
    edges with value dinv^2, Cheb normalization negated so the scatter
    directly produces lhat).
  * AllGather overlap: sources are additionally split into two HALVES by
    node range (rows 0-639 / 640-1279 of each rank). Each replicated
    feature tensor lives as two DRAM tensors (one per half) AllGathered
    separately: the half-0 AG is issued when local tiles 0-4 finish, so the
    next pass's half-0 gathers/matmuls (a full sweep over all 10 dest
    tiles, accumulated to SBUF partials) overlap the half-1 AG flight.
    Sweep B then adds the partial back (identity matmul) plus the half-1
    scatter.
  * ChebConv K=3 exploits linearity of lhat (lhat(Z)@W == lhat(Z@W)):
      out = celu(h2@Wa + lhat(h2@Wb) + lhat(lhat(h2@Wc2)) + bc)
    with Wa=Wk0-Wk2, Wb=Wk1, Wc2=2*Wk2. BC = h2@[Wb|Wc2] is computed
    locally and AllGathered (512 wide); pass C1 aggregates BC giving
    [lhatB | lhatC]; lhatC is AllGathered (256 wide, half cost) and pass
    C2 aggregates it at 256 width, accumulating lhatB (via identity
    matmul) and h2@Wa into the same PSUM.
"""
import numpy as np
import ml_dtypes

import concourse.bacc as bacc
import concourse.mybir as mybir
import concourse.tile as tile
from concourse import library_config
from concourse.bass_utils import run_bass_kernel_spmd
from concourse.tile import add_dep_helper

BF16 = ml_dtypes.bfloat16
FP32 = mybir.dt.float32
BF16D = mybir.dt.bfloat16
I16 = mybir.dt.int16

P = 8            # cores
N = 10000        # nodes
NPC = N // P     # nodes per core
NPAD = 1280      # padded nodes per core
F = 512          # feature width of x / h1 / h2
DOUT = 256
DT = 128         # dests per dest tile
NDT = NPAD // DT # dest tiles per core
KC = F // 128    # contraction chunks (4)
NCH = 2          # AllGather chunks (source halves) per tensor
CH = NPAD // NCH # local rows per AG chunk (640)
HALF = P * CH    # rows per half tensor (5120)


# ----------------------------------------------------------------- host prep

def _half_of(n):
    return (n % NPC) // CH


def _half_id(n):
    """Global node id -> row within its half tensor [P ranks][CH rows]."""
    return (n // NPC) * CH + (n % NPC) % CH


def _build_edge_tiles(src, dst, val):
    """Shard by dest core, tile by 128 dests, split sources by half, dedup
    per (tile, half). Returns (ETF [2*NDT], idx [P, T, 128], S [P,T,128,DT])
    with flat units ordered (t0,h0),(t0,h1),(t1,h0),..."""
    per_core = []
    order = np.argsort(dst, kind="stable")
    src, dst, val = src[order], dst[order], val[order]
    core_of = dst // NPC
    core_starts = np.searchsorted(core_of, np.arange(P + 1))
    for c in range(P):
        lo, hi = core_starts[c], core_starts[c + 1]
        s, d, v = src[lo:hi], dst[lo:hi] - c * NPC, val[lo:hi]
        tile_of = d // DT
        tile_starts = np.searchsorted(tile_of, np.arange(NDT + 1))
        groups = []
        for t in range(NDT):
            a, b = tile_starts[t], tile_starts[t + 1]
            st, dl, vt = s[a:b], d[a:b] - t * DT, v[a:b]
            hf = _half_of(st)
            for j in (0, 1):
                m = hf == j
                sj, dj, vj = st[m], dl[m], vt[m]
                uniq, inv = np.unique(sj, return_inverse=True)
                if len(uniq) == 0:
                    groups.append((np.zeros(0, np.int64),
                                   np.zeros((0, DT), np.float32)))
                    continue
                S = np.zeros((len(uniq), DT), np.float32)
                np.add.at(S, (inv, dj), vj)
                groups.append((uniq, S))
        per_core.append(groups)

    NU = 2 * NDT
    ETF = [max(max((len(per_core[c][u][0]) + 127) // 128, 1) for c in range(P))
           for u in range(NU)]
    ROWS = [max(max(len(per_core[c][u][0]), 1) for c in range(P))
            for u in range(NU)]
    T = sum(ETF)
    off = np.cumsum([0] + ETF[:-1])
    idx = np.zeros((P, T, 128), np.int32)
    S_all = np.zeros((P, T, 128, DT), np.float32)
    for c in range(P):
        for u in range(NU):
            uniq, S = per_core[c][u]
            n = len(uniq)
            o = off[u]
            if n == 0:
                continue
            idx[c, o:o + (n + 127) // 128].reshape(-1)[:n] = _half_id(uniq)
            S_all[c, o:o + (n + 127) // 128].reshape(-1, DT)[:n] = S
    uniq_lists = [[per_core[c][u][0] for u in range(NU)] for c in range(P)]
    return (tuple(ETF), tuple(ROWS)), idx, S_all, uniq_lists


def _idx_dev(idx_core):
    """[T, 128] int32 -> [128, T*8] int16 (wrap 16 partitions, replicate x8)."""
    flat = idx_core.reshape(-1)
    n = len(flat)
    a = np.zeros((16, n // 16), np.int16)
    a[np.arange(n) % 16, np.arange(n) // 16] = flat.astype(np.int16)
    return np.tile(a, (8, 1))


def _s_dev(S_core):
    """[T, 128, DT] -> [128, T*DT] bf16."""
    T = S_core.shape[0]
    return np.ascontiguousarray(
        S_core.transpose(1, 0, 2).reshape(128, T * DT)).astype(BF16)


def _w_dev(W):
    """[F, fo] -> [128, KC*fo] bf16 (chunk k at cols [k*fo, (k+1)*fo))."""
    fi, fo = W.shape
    k = fi // 128
    return np.ascontiguousarray(
        W.reshape(k, 128, fo).transpose(1, 0, 2).reshape(128, k * fo)).astype(BF16)


def _prep(x, edge_index, edge_weight, W1, b1, W2, b2, Wc, bc):
    row = np.asarray(edge_index[0], np.int64)
    col = np.asarray(edge_index[1], np.int64)
    w = np.asarray(edge_weight, np.float32)

    # GCN norm (layers 1 & 2): deg over dest (col) + 1 self loop.
    deg = np.zeros(N, np.float32)
    np.add.at(deg, col, w)
    deg += 1.0
    dinv = (1.0 / np.sqrt(deg)).astype(np.float32)
    g_src = np.concatenate([row, np.arange(N)])
    g_dst = np.concatenate([col, np.arange(N)])
    g_val = np.concatenate([dinv[row] * w * dinv[col], dinv * dinv]).astype(np.float32)

    # Cheb: drop self loops, deg over src (row), negate (lhat = -A_norm).
    keep = row != col
    r0, c0, w0 = row[keep], col[keep], w[keep]
    deg2 = np.zeros(N, np.float32)
    np.add.at(deg2, r0, w0)
    dinv2 = np.where(deg2 > 0, 1.0 / np.sqrt(deg2), 0.0).astype(np.float32)
    c_val = -(dinv2[r0] * w0 * dinv2[c0]).astype(np.float32)

    ETg, idxg, Sg, ulg = _build_edge_tiles(g_src, g_dst, g_val)
    ETc, idxc, Sc, _ = _build_edge_tiles(r0, c0, c_val)

    xbf = np.asarray(x, np.float32).astype(BF16)
    TGc = sum(ETg[0])
    offu = np.cumsum([0] + list(ETg[0][:-1]))
    xms = []
    for c in range(P):
        arr = np.zeros((128, TGc, F), BF16)
        for u in range(2 * NDT):
            uniq = ulg[c][u]
            n = len(uniq)
            if n == 0:
                continue
            j = np.arange(n)
            arr[j % 128, offu[u] + j // 128] = xbf[uniq]
        xms.append(arr)

    Wc = np.asarray(Wc, np.float32)
    wbc = np.concatenate([Wc[1], 2.0 * Wc[2]], axis=1)   # [512, 512] = [Wb|Wc2]
    com = dict(
        w1=_w_dev(np.asarray(W1, np.float32)),
        w2=_w_dev(np.asarray(W2, np.float32)),
        wa=_w_dev(Wc[0] - Wc[2]),
        wbc=_w_dev(wbc),
        ident=np.eye(128, dtype=BF16),
    )
    biases = (np.asarray(b1, np.float32), np.asarray(b2, np.float32),
              np.asarray(bc, np.float32))
    in_maps = []
    for c in range(P):
        m = dict(com)
        m["xm"] = xms[c]
        m["idxg"] = _idx_dev(idxg[c])
        m["sg"] = _s_dev(Sg[c])
        m["idxc"] = _idx_dev(idxc[c])
        m["sc"] = _s_dev(Sc[c])
        in_maps.append(m)
    return ETg, ETc, biases, in_maps


# ------------------------------------------------------------- bass program

_CACHE = {}


def _build_program(ETg, ETc, has_bias):
    import os
    key = (ETg, ETc, has_bias, os.environ.get("GNN_PHASES", "9"))
    if key in _CACHE:
        return _CACHE[key]
    ETg, RWg = ETg
    ETc, RWc = ETc
    TG, TC = sum(ETg), sum(ETc)
    ETMAX = max(max(ETg), max(ETc))

    nc = bacc.Bacc("TRN2", target_bir_lowering=False, num_devices=P,
                   num_swdge_queues=4)
    xm = nc.dram_tensor("xm", [128, TG, F], BF16D, kind="ExternalInput")
    idxg = nc.dram_tensor("idxg", [128, TG * 8], I16, kind="ExternalInput")
    sg = nc.dram_tensor("sg", [128, TG * DT], BF16D, kind="ExternalInput")
    idxc = nc.dram_tensor("idxc", [128, TC * 8], I16, kind="ExternalInput")
    sc = nc.dram_tensor("sc", [128, TC * DT], BF16D, kind="ExternalInput")
    w1 = nc.dram_tensor("w1", [128, KC * F], BF16D, kind="ExternalInput")
    w2 = nc.dram_tensor("w2", [128, KC * F], BF16D, kind="ExternalInput")
    wa = nc.dram_tensor("wa", [128, KC * DOUT], BF16D, kind="ExternalInput")
    wbc = nc.dram_tensor("wbc", [128, KC * F], BF16D, kind="ExternalInput")
    ident = nc.dram_tensor("ident", [128, 128], BF16D, kind="ExternalInput")
    if has_bias:
        brows = nc.dram_tensor("brows", [1, 2 * F + DOUT], FP32, kind="ExternalInput")
    outp = nc.dram_tensor("out", [NPAD, DOUT], FP32, kind="ExternalOutput")

    h1c = nc.dram_tensor("h1c", [NPAD, F], BF16D, kind="Internal")
    h1fa = nc.dram_tensor("h1fa", [HALF, F], BF16D, kind="Internal",
                          addr_space="Shared")
    h1fb = nc.dram_tensor("h1fb", [HALF, F], BF16D, kind="Internal",
                          addr_space="Shared")
    bcc = nc.dram_tensor("bcc", [NPAD, F], BF16D, kind="Internal")
    bcfa = nc.dram_tensor("bcfa", [HALF, F], BF16D, kind="Internal",
                          addr_space="Shared")
    bcfb = nc.dram_tensor("bcfb", [HALF, F], BF16D, kind="Internal",
                          addr_space="Shared")
    lcc = nc.dram_tensor("lcc", [NPAD, DOUT], BF16D, kind="Internal")
    lcfa = nc.dram_tensor("lcfa", [HALF, DOUT], BF16D, kind="Internal",
                          addr_space="Shared")
    lcfb = nc.dram_tensor("lcfb", [HALF, DOUT], BF16D, kind="Internal",
                          addr_space="Shared")

    Exp = mybir.ActivationFunctionType.Exp
    Alu = mybir.AluOpType

    with tile.TileContext(nc) as tc:
        with (
            tc.tile_pool(name="const", bufs=1) as cpool,
            tc.tile_pool(name="keep", bufs=1) as kpool,
            tc.tile_pool(name="msgs", bufs=6) as mpool,
            tc.tile_pool(name="msgs2", bufs=3) as mpool2,
            tc.tile_pool(name="work", bufs=2) as wpool,
            tc.tile_pool(name="psum", bufs=2, space="PSUM") as ppool,
            tc.tile_pool(name="psum3", bufs=3, space="PSUM") as ppool3,
        ):
            lib = nc.gpsimd.load_library(library_config.mlp)

            ig_sb = cpool.tile([128, TG * 8], I16, tag="ig")
            nc.sync.dma_start(ig_sb[:], idxg[:])
            ic_sb = cpool.tile([128, TC * 8], I16, tag="ic")
            nc.sync.dma_start(ic_sb[:], idxc[:])
            id_sb = cpool.tile([128, 128], BF16D, tag="id")
            nc.sync.dma_start(id_sb[:], ident[:])

            SMAX = max(TG, TC)
            s_sb = cpool.tile([128, SMAX * DT], BF16D, tag="s")
            offg = np.cumsum([0] + list(ETg[:-1]))
            offc = np.cumsum([0] + list(ETc[:-1]))
            for u in range(2 * NDT):
                a, b = offg[u] * DT, (offg[u] + ETg[u]) * DT
                nc.sync.dma_start(s_sb[:, a:b], sg[:, a:b])

            w1_sb = cpool.tile([128, KC * F], BF16D, tag="w1")
            nc.sync.dma_start(w1_sb[:], w1[:])
            w2_sb = cpool.tile([128, KC * F], BF16D, tag="w2")
            nc.sync.dma_start(w2_sb[:], w2[:])
            wa_sb = cpool.tile([128, KC * DOUT], BF16D, tag="wa")
            nc.sync.dma_start(wa_sb[:], wa[:])
            wbc_sb = cpool.tile([128, KC * F], BF16D, tag="wbc")
            nc.sync.dma_start(wbc_sb[:], wbc[:])
            if has_bias:
                br_sb = cpool.tile([1, 2 * F + DOUT], FP32, tag="br")
                nc.sync.dma_start(br_sb[:], brows[:])
                ones_sb = cpool.tile([1, 128], FP32, tag="ones")
                nc.vector.memset(ones_sb[:], 1.0)

            for _i in range(6):
                mt = mpool.tile([128, ETMAX, F], BF16D, tag="msgs")
                nc.vector.memset(mt[:], 0)
            for _i in range(3):
                mt2 = mpool2.tile([128, ETMAX, DOUT], BF16D, tag="msgs2")
                nc.vector.memset(mt2[:], 0)

            h2keep = kpool.tile([128, NDT, F], BF16D, tag="h2k")
            aggTk = kpool.tile([128, NDT, F], BF16D, tag="aTk")
            lbkeep = kpool.tile([128, NDT, DOUT], BF16D, tag="lbk")
            part = kpool.tile([128, NDT, F], BF16D, tag="part")

            first_gather = [0]
            qctr = [0]

            def scatter_unit(src_dram, u, idx_sb, s_sb, off, ET, RW, width, ps,
                             start, stop, fm=False):
                """Gather + one-hot matmuls for flat unit u into psum ap `ps`."""
                o = off[u]
                et = ET[u]
                rows = RW[u]
                if width == F:
                    msgs = mpool.tile([128, ETMAX, F], BF16D, tag="msgs")
                else:
                    msgs = mpool2.tile([128, ETMAX, DOUT], BF16D, tag="msgs2")
                q = qctr[0] % 4
                qctr[0] += 1
                gi = nc.gpsimd.dma_gather(
                    msgs[:, :et, :width], src_dram[:],
                    idx_sb[:, o * 8:(o + et) * 8],
                    rows, rows, width,
                    single_packet=False, queue_num=q)
                if first_gather[0] < 4:
                    add_dep_helper(gi.ins, lib.ins,
                                   reason="mlp lib before gather")
                    first_gather[0] += 1
                if fm:
                    for g in range(et):
                        mm_fm(ps, msgs, g, o, s_sb, start, stop, et - 1)
                else:
                    for g in range(et):
                        nc.tensor.matmul(
                            ps,
                            s_sb[:, (o + g) * DT:(o + g + 1) * DT],
                            msgs[:, g, :width],
                            start=(start and g == 0),
                            stop=(stop and g == et - 1))

            def celu(z_ps, width, out_ap):
                """out = max(z,0) + min(exp(z)-1, 0); z read from PSUM."""
                e = wpool.tile([128, F], FP32, tag="e")
                nc.scalar.activation(e[:, :width], z_ps, Exp)
                em = wpool.tile([128, F], FP32, tag="em")
                nc.vector.tensor_scalar(
                    em[:, :width], e[:, :width], 1.0, 0.0,
                    Alu.subtract, Alu.min)
                nc.vector.scalar_tensor_tensor(
                    out_ap, z_ps, 0.0, em[:, :width], Alu.max, Alu.add)

            def gemm_bias(z_ps, width, b_off):
                if has_bias:
                    nc.tensor.matmul(
                        z_ps, ones_sb[:],
                        br_sb[:, b_off:b_off + width],
                        start=False, stop=False)

            def allgather_half(cin, cout, j):
                nc.gpsimd.collective_compute(
                    "AllGather", Alu.bypass,
                    replica_groups=[list(range(P))],
                    ins=[cin[j * CH:(j + 1) * CH, :]],
                    outs=[cout[:]])

            def mm_fm(psA, msgs, g, o, s_sb_, start, stop, last):
                """Feature-major scatter matmuls for edge chunk g:
                psA[f_k, d] += msgs_k.T @ S  (msgs chunk is stationary).
                start/stop fire once per PSUM tile: the 2KB zero-region
                covers all four k slices."""
                for k in range(KC):
                    nc.tensor.matmul(
                        psA[:, k, :],
                        msgs[:, g, k * 128:(k + 1) * 128],
                        s_sb_[:, (o + g) * DT:(o + g + 1) * DT],
                        start=(start and g == 0 and k == 0),
                        stop=(stop and g == last and k == KC - 1))

            def load_unit(u, psA, start, stop):
                """Host-pregathered L1 messages: bulk HWDGE load, no desc-gen."""
                o = offg[u]
                et = ETg[u]
                msgs = mpool.tile([128, ETMAX, F], BF16D, tag="msgs")
                nc.sync.dma_start(msgs[:, :et, :], xm[:, o:o + et, :])
                for g in range(et):
                    mm_fm(psA, msgs, g, o, s_sb, start, stop, et - 1)

            # ---- layer 1: h1 = celu((Ag @ x) @ W1 + b1)  (x replicated;
            #      messages pre-gathered on host). Scatter and GEMM run as
            #      separate sweeps (no PE stall on the DVE aggregate copy),
            #      interleaved by half-groups so the AGs fire early.
            def l1_s1(ts):
                for t in ts:
                    psA = ppool3.tile([128, KC, 128], FP32, tag="psA")
                    load_unit(2 * t, psA, True, False)
                    load_unit(2 * t + 1, psA, False, True)
                    nc.vector.tensor_copy(aggTk[:, t, :], psA[:])

            def l1_s2(ts):
                for t in ts:
                    z = ppool.tile([128, F], FP32, tag="z")
                    for k in range(KC):
                        nc.tensor.matmul(
                            z[:], aggTk[:, t, k * 128:(k + 1) * 128],
                            w1_sb[:, k * F:(k + 1) * F],
                            start=(k == 0), stop=(k == KC - 1))
                    gemm_bias(z[:], F, 0)
                    h = wpool.tile([128, F], BF16D, tag="h")
                    celu(z[:], F, h[:])
                    nc.scalar.dma_start(h1c[t * 128:(t + 1) * 128, :], h[:])

            l1_s1(range(0, 5))
            l1_s2(range(0, 5))
            allgather_half(h1c, h1fa, 0)
            l1_s1(range(5, NDT))
            l1_s2(range(5, NDT))
            allgather_half(h1c, h1fb, 1)

            # ---- layer 2: h2 = celu((Ag @ h1) @ W2 + b2); kept on chip
            # (feature-major throughout: no transposes; h2keep holds h2T).
            # Sweep A: half-0 partial aggregates -> SBUF (overlaps h1fb AG).
            for t in range(NDT):
                psA = ppool3.tile([128, KC, 128], FP32, tag="psA")
                scatter_unit(h1fa, 2 * t, ig_sb, s_sb, offg, ETg, RWg, F, psA,
                             start=True, stop=True, fm=True)
                nc.vector.tensor_copy(part[:, t, :], psA[:])
            # Sweep B1: half-1 scatter + partial add -> aggT (SBUF).
            def l2_b1(ts):
                for t in ts:
                    psA = ppool3.tile([128, KC, 128], FP32, tag="psA")
                    for k in range(KC):
                        nc.tensor.matmul(psA[:, k, :], id_sb[:],
                                         part[:, t, k * 128:(k + 1) * 128],
                                         start=(k == 0), stop=False)
                    scatter_unit(h1fb, 2 * t + 1, ig_sb, s_sb, offg, ETg, RWg,
                                 F, psA, start=False, stop=True, fm=True)
                    nc.vector.tensor_copy(aggTk[:, t, :], psA[:])
            # Sweep B2: W-stationary GEMM -> zT (feature-major), celu -> h2T,
            # then BC = h2 @ [Wb | 2*Wc2] -> store + AG (same sweep so the
            # bcf AllGathers fire as early as possible).
            def l2_b2(ts):
              for t in ts:
                zt = ppool.tile([128, F], FP32, tag="z")
                for j in range(KC):
                    for k in range(KC):
                        nc.tensor.matmul(
                            zt[:, j * 128:(j + 1) * 128],
                            w2_sb[:, k * F + j * 128:k * F + (j + 1) * 128],
                            aggTk[:, t, k * 128:(k + 1) * 128],
                            start=(j == 0 and k == 0),
                            stop=(j == KC - 1 and k == KC - 1))
                    if has_bias:
                        nc.tensor.matmul(
                            zt[:, j * 128:(j + 1) * 128],
                            br_sb[:, F + j * 128:F + (j + 1) * 128],
                            ones_sb[:, :128],
                            start=False, stop=False)
                celu(zt[:], F, h2keep[:, t, :])
                zb = ppool.tile([128, F], FP32, tag="z")
                for k in range(KC):
                    nc.tensor.matmul(
                        zb[:], h2keep[:, t, k * 128:(k + 1) * 128],
                        wbc_sb[:, k * F:(k + 1) * F],
                        start=(k == 0), stop=(k == KC - 1))
                bcn = wpool.tile([128, F], BF16D, tag="h")
                nc.vector.tensor_copy(bcn[:], zb[:])
                nc.scalar.dma_start(bcc[t * 128:(t + 1) * 128, :], bcn[:])

            l2_b1(range(0, 5))
            l2_b2(range(0, 5))
            allgather_half(bcc, bcfa, 0)
            l2_b1(range(5, NDT))
            l2_b2(range(5, NDT))
            allgather_half(bcc, bcfb, 1)

            # ---- reload shared S buffer with the cheb S matrices (WAR dep
            # on all layer-1/2 matmuls orders this after their last use)
            nc.sync.dma_start(s_sb[:, :TC * DT], sc[:])

            # ---- cheb pass C1: [lhatB | lhatC] = lhat(BC)
            for t in range(NDT):
                ps = ppool3.tile([128, F], FP32, tag="psT")
                scatter_unit(bcfa, 2 * t, ic_sb, s_sb, offc, ETc, RWc, F, ps[:],
                             start=True, stop=True)
                nc.vector.tensor_copy(part[:, t, :], ps[:])
            for t in range(NDT):
                ps = ppool3.tile([128, F], FP32, tag="psT")
                nc.tensor.matmul(ps[:], id_sb[:], part[:, t, :],
                                 start=True, stop=False)
                scatter_unit(bcfb, 2 * t + 1, ic_sb, s_sb, offc, ETc, RWc, F,
                             ps[:], start=False, stop=True)
                nc.vector.tensor_copy(lbkeep[:, t, :], ps[:, :DOUT])
                lcn = wpool.tile([128, DOUT], BF16D, tag="lc")
                nc.vector.tensor_copy(lcn[:], ps[:, DOUT:])
                nc.scalar.dma_start(lcc[t * 128:(t + 1) * 128, :], lcn[:])
                if t == 4:
                    allgather_half(lcc, lcfa, 0)
                if t == 9:
                    allgather_half(lcc, lcfb, 1)

            # ---- cheb pass C2 + output:
            # out = celu(lhat(lhatC) + lhatB + h2 @ Wa + bc)
            for t in range(NDT):
                zt = ppool3.tile([128, F], FP32, tag="psT")
                scatter_unit(lcfa, 2 * t, ic_sb, s_sb, offc, ETc, RWc, DOUT,
                             zt[:, :DOUT], start=True, stop=True)
                nc.vector.tensor_copy(part[:, t, :DOUT], zt[:, :DOUT])
            for t in range(NDT):
                zt = ppool3.tile([128, F], FP32, tag="psT")
                zo = zt[:, :DOUT]
                nc.tensor.matmul(zo, id_sb[:], part[:, t, :DOUT],
                                 start=True, stop=False)
                scatter_unit(lcfb, 2 * t + 1, ic_sb, s_sb, offc, ETc, RWc, DOUT,
                             zo, start=False, stop=False)
                nc.tensor.matmul(zo, id_sb[:], lbkeep[:, t, :],
                                 start=False, stop=False)
                for k in range(KC):
                    nc.tensor.matmul(
                        zo, h2keep[:, t, k * 128:(k + 1) * 128],
                        wa_sb[:, k * DOUT:(k + 1) * DOUT],
                        start=False, stop=(k == KC - 1))
                gemm_bias(zo, DOUT, 2 * F)
                of = wpool.tile([128, DOUT], FP32, tag="of")
                celu(zo, DOUT, of[:])
                nc.scalar.dma_start(outp[t * 128:(t + 1) * 128, :], of[:])

    nc.compile()
    _CACHE[key] = nc
    return nc


# ------------------------------------------------------------------- driver

def _run(inputs, trace=False, tmpdir=None):
    ETg, ETc, biases, in_maps = _prep(**inputs)
    has_bias = any(np.any(b != 0) for b in biases)
    if has_bias:
        brow = np.concatenate(biases).astype(np.float32)[None, :]
        for m in in_maps:
            m["brows"] = brow
    nc = _build_program(ETg, ETc, has_bias)
    res = run_bass_kernel_spmd(nc, in_maps, core_ids=list(range(P)),
                               trace=trace, tmpdir=tmpdir)
    out = np.concatenate(
        [res.results[c]["out"][:NPC] for c in range(P)], axis=0)
    return out.astype(np.float32), res


def kernel(**inputs) -> np.ndarray:
    out, _ = _run(inputs)
    return out
